# revision 14
# baseline (speedup 1.0000x reference)
"""Trainium2 Bass kernel for ViT-style LSA attention (sparse_attention).

Reference computation (per batch item):
    qkv = x @ W_qkv.T ; split q,k,v into 12 heads of 64
    dots = (q @ k.T) * scale[h]; diagonal masked to -inf; softmax
    out = (attn @ v) reassembled, then @ W_out.T + b_out

Sharding: data-parallel over batch across 8 NeuronCores (8 items each).

Per-core dataflow (all matmuls bf16 with fp32 PSUM accumulation):
  - x^T and W^T tiles produced on-chip: paired-row DMA loads (fp32) ->
    DVE convert to bf16 -> one DMA xbar transpose per [128, 768] tile.
  - qk^T = W_qk^T.T @ x^T (feature-major); per-head LSA scale folded
    into the Q tiles during the PSUM->SBUF drain.
  - V natural = x^T.T @ Wv^T stored per head in 65-wide blocks whose
    65th column is 1.0 so the PV matmul also produces softmax row-sums.
  - S^T per (item, head-pair): the pair's two S^T results go to TWO
    separate PSUM banks of one padded tile; one wide exp covers both
    heads; multiplicative diagonal mask applied post-exp on DVE.
  - out^T (+rowsum row 64) = V_aug.T @ P^T ; normalize with DVE
    reciprocal + gpsimd partition_broadcast + DVE mult into attn^T.
  - final = attn^T.T @ W_out^T; b_out added during the DVE PSUM drain.

Scheduling: the whole per-core program is emitted as an item-pipelined
interleave. Attention chains are software-pipelined (S of head-pair
h+2 emitted before PV of head-pair h) and independent "filler" units
(qkT chunk pieces, V items, out-projections of the previous item) are
pulled from a queue between the S/PV matmuls so the tensor engine
never waits on the scalar/DVE softmax chain (which would also drop its
DVFS p-state). PSUM budget is exactly 8 banks: psA 2 (projection
fills), psS 2x2 (S tiles, head-pair wide), psO 1x2 (PV accumulators).

HW notes (verified the hard way): two matmul accumulation groups may
NOT share a PSUM bank (runtime crash, also with a single start/stop
spanning disjoint ranges); custom-DVE ops (reciprocal_approx_fast)
crash at runtime in this axon environment.
"""

from collections import deque
from contextlib import ExitStack

import numpy as np
import ml_dtypes

import concourse.bass as bass
import concourse.bacc as bacc
import concourse.mybir as mybir
import concourse.tile as tile
from concourse import bass_utils, library_config

F32 = mybir.dt.float32
BF16 = mybir.dt.bfloat16
AF = mybir.ActivationFunctionType
ALU = mybir.AluOpType

B, N, D, H, DH = 64, 197, 768, 12, 64
NCORES = 8
BPC = B // NCORES  # batch items per core
KT = D // 128      # 6 contraction tiles of 128
NT_QK = (2 * D) // 128  # 12 feature tiles for q,k


def build_nc(bpc=BPC, repeat=1):
    """Build the kernel. repeat>1 emits the whole body N times back-to-back
    (used only for timing: differencing two repeat counts cancels the fixed
    PJRT dispatch + host<->device transfer overhead)."""
    M = bpc * N  # tokens per core

    nc = bacc.Bacc("TRN2", target_bir_lowering=False, debug=False,
                   num_devices=NCORES)
    x_d = nc.dram_tensor("x", [bpc, N, D], F32, kind="ExternalInput")
    wqkv_d = nc.dram_tensor("w_qkv", [3 * D, D], F32, kind="ExternalInput")
    scale_d = nc.dram_tensor("scale", [H], F32, kind="ExternalInput")
    wout_d = nc.dram_tensor("w_out", [D, D], F32, kind="ExternalInput")
    bout_d = nc.dram_tensor("b_out", [D], F32, kind="ExternalInput")
    out_d = nc.dram_tensor("out", [bpc, N, D], F32, kind="ExternalOutput")

    # Multiplicative diagonal mask for P^T tiles: mask[p, jt, i] = 0 iff
    # i == jt*128 + p (the self-attention position), else 1.
    mask_np = np.ones((128, 2, 2 * N), dtype=ml_dtypes.bfloat16)
    for jt in range(2):
        for p in range(128):
            i = jt * 128 + p
            if i < N:
                mask_np[p, jt, i] = 0
                mask_np[p, jt, N + i] = 0
    mask_d = nc.inline_tensor(mask_np, name="maskc")

    x_flat = x_d[:, :, :].flatten_outer_dims()  # [M, D]
    jtiles = [(0, 128), (1, N - 128)]

    with tile.TileContext(nc) as tc, ExitStack() as es:
        res = es.enter_context(tc.tile_pool(name="res", bufs=1))

        nc.gpsimd.load_library(library_config.attn)

        # ---- resident tiles (allocated once, written by each repeat) ----
        mask_sb = res.tile([128, 2, 2 * N], BF16, name="mask_sb")
        scale_row = res.tile([1, H], F32, name="scale_row")
        scale_bc = res.tile([128, H], F32, name="scale_bc")
        scale_bc2 = res.tile([128, KT, 1], F32, name="scale_bc2")
        brow = res.tile([1, D], F32, name="brow")
        bias_bc = res.tile([128, D], F32, name="bias_bc")
        # token dim padded to 128 so the xbar transpose always moves full
        # [128, 128] tiles (row count must be a multiple of 16); the padding
        # is zero-filled and never read by any matmul.
        M_pad = ((M + 127) // 128) * 128
        xT = res.tile([128, KT, M_pad], BF16, name="xT")
        wqkT = res.tile([128, KT, 2 * D], BF16, name="wqkT")
        wvT = res.tile([128, KT, D], BF16, name="wvT")
        woT = res.tile([128, KT, D], BF16, name="woT")
        qkT = res.tile([128, NT_QK, M], BF16, name="qkT")
        v_sb = res.tile([128, bpc, 2, H * 65], BF16, name="v_sb")

        # ---- pools ----
        stg = es.enter_context(tc.tile_pool(name="stg", bufs=4))
        stgb = es.enter_context(tc.tile_pool(name="stgb", bufs=4))
        # PSUM pools (8 banks total: psA 2x1, psS 2x2, psO 1x2)
        psA = es.enter_context(tc.tile_pool(name="psA", bufs=2, space="PSUM"))
        psS = es.enter_context(tc.tile_pool(name="psS", bufs=2, space="PSUM"))
        psO = es.enter_context(tc.tile_pool(name="psO", bufs=1, space="PSUM"))
        ptp = es.enter_context(tc.tile_pool(name="ptp", bufs=8))
        rcp = es.enter_context(tc.tile_pool(name="rcp", bufs=3))
        bcp = es.enter_context(tc.tile_pool(name="bcp", bufs=3))
        atp = es.enter_context(tc.tile_pool(name="atp", bufs=3))
        osp = es.enter_context(tc.tile_pool(name="osp", bufs=3))

        def stage_waves(jobs, wave=4):
            """jobs: list of (src2_ap, nrows_pair, (dst_3d, dst_3d|None)).
            src2_ap covers up to 2 row-tiles as [128, n2, D] (row t*128+p at
            [p, t, :]). Waves: all loads, all converts, then all transposes —
            grouping transposes avoids DMACopy<->DMATranspose xbar-mode
            serialization on the HWDGE path."""
            for w0 in range(0, len(jobs), wave):
                batch = jobs[w0:w0 + wave]
                tbs = []
                for src_ap, nrows, dsts in batch:
                    n2 = len([d for d in dsts if d is not None])
                    t_f = stg.tile([128, 2, D], F32, tag="stg", name="t_f")
                    if nrows < n2 * 128:
                        nc.vector.memset(t_f, 0.0)
                    if nrows > 128:
                        nc.sync.dma_start(
                            t_f[:, :2],
                            src_ap.rearrange("(t p) f -> p t f", p=128))
                    else:
                        nc.sync.dma_start(t_f[:nrows, 0], src_ap)
                    t_b = stgb.tile([128, 2, D], BF16, tag="stgb", name="t_b")
                    nc.vector.tensor_copy(t_b[:, :n2], t_f[:, :n2])
                    tbs.append(t_b)
                for (src_ap, nrows, dsts), t_b in zip(batch, tbs):
                    for t, dst in enumerate(dsts):
                        if dst is None:
                            continue
                        # one xbar transpose per [128, 768] tile: 3D dst
                        # gets row kt*128+p at [p, kt, m] (sim-verified)
                        nc.sync.dma_start_transpose(dst, t_b[:, t])

        def pair_jobs(src_rows, total_rows, dst_fn):
            jobs = []
            r0 = 0
            while r0 < total_rows:
                rows = min(256, total_rows - r0)
                if rows < 256:
                    rows = min(128, rows)  # singles for the tail
                d0 = dst_fn(r0)
                d1 = dst_fn(r0 + 128) if rows > 128 else None
                jobs.append((src_rows(r0, rows), rows, (d0, d1)))
                r0 += rows
            return jobs

        def emit_setup():
            nc.sync.dma_start(mask_sb, mask_d[:, :, :])
            nc.sync.dma_start(scale_row, scale_d[None, :])
            nc.gpsimd.partition_broadcast(scale_bc, scale_row)
            for nt in range(KT):
                for hh in range(2):
                    nc.vector.tensor_copy(
                        scale_bc2[hh * 64:(hh + 1) * 64, nt],
                        scale_bc[hh * 64:(hh + 1) * 64,
                                 2 * nt + hh:2 * nt + hh + 1])
            nc.sync.dma_start(brow, bout_d[None, :])
            nc.gpsimd.partition_broadcast(bias_bc, brow)
            # DMA priority: W_qk and x interleaved (gate qkT chunk 0),
            # then Wv (gates V items), W_out last (gates only outproj).
            jw = pair_jobs(lambda r0, rows: wqkv_d[r0:r0 + rows, :],
                           2 * D, lambda r0: wqkT[:, :, r0:r0 + 128])
            jx = pair_jobs(lambda r0, rows: x_flat[r0:r0 + rows, :],
                           M, lambda r0: xT[:, :, r0:r0 + 128])
            jv = pair_jobs(
                lambda r0, rows: wqkv_d[2 * D + r0:2 * D + r0 + rows, :],
                D, lambda r0: wvT[:, :, r0:r0 + 128])
            jo = pair_jobs(lambda r0, rows: wout_d[r0:r0 + rows, :],
                           D, lambda r0: woT[:, :, r0:r0 + 128])
            jobs = [jx[0], jx[1], jw[0], jw[1], jx[2], jw[2], jw[3],
                    jx[3], jw[4], jw[5], jx[4], jx[5], jx[6]]
            jobs += jv + jo
            stage_waves(jobs)

        # ---------- filler units (each emits a few PE ops + drain) ----------
        def unit_qkT(nt, mc0):
            def emit():
                mcs = min(512, M - mc0)
                ps = psA.tile([128, 512], F32, tag="psA", name="ps_qk")
                for kt in range(KT):
                    nc.tensor.matmul(
                        ps[:, :mcs],
                        lhsT=wqkT[:, kt, nt * 128:(nt + 1) * 128],
                        rhs=xT[:, kt, mc0:mc0 + mcs],
                        start=(kt == 0), stop=(kt == KT - 1))
                if nt < KT:  # Q tiles: fold in the per-head LSA scale
                    nc.scalar.mul(qkT[:, nt, mc0:mc0 + mcs], ps[:, :mcs],
                                  scale_bc2[:, nt])
                else:
                    nc.scalar.copy(qkT[:, nt, mc0:mc0 + mcs], ps[:, :mcs])
            return emit

        def unit_v(b, jt, jsz, nch):
            def emit():
                ps = psA.tile([128, 512], F32, tag="psA", name="ps_v")
                for kt in range(KT):
                    nc.tensor.matmul(
                        ps[:jsz, :384],
                        lhsT=xT[:, kt,
                                b * N + jt * 128:b * N + jt * 128 + jsz],
                        rhs=wvT[:, kt, nch * 384:(nch + 1) * 384],
                        start=(kt == 0), stop=(kt == KT - 1))
                dst = v_sb[:jsz, b, jt].rearrange(
                    "p (h e) -> p h e", e=65)[:, nch * 6:(nch + 1) * 6, 0:64]
                nc.scalar.copy(
                    dst, ps[:jsz, :384].rearrange("p (h e) -> p h e", e=64))
            return emit

        def unit_outproj(b, jt, jsz, attnT):
            def emit():
                osb = osp.tile([128, D], F32, tag="osb", name="osb")
                for nch in range(2):
                    ps = psA.tile([128, 512], F32, tag="psA", name="ps_o")
                    for ft in range(KT):
                        nc.tensor.matmul(
                            ps[:jsz, :384],
                            lhsT=attnT[:, ft, jt * 128:jt * 128 + jsz],
                            rhs=woT[:, ft, nch * 384:(nch + 1) * 384],
                            start=(ft == 0), stop=(ft == KT - 1))
                    # drain with the output bias added (b_out broadcast)
                    nc.vector.tensor_tensor(
                        osb[:jsz, nch * 384:(nch + 1) * 384],
                        ps[:jsz, :384],
                        bias_bc[:jsz, nch * 384:(nch + 1) * 384],
                        op=ALU.add)
                nc.sync.dma_start(out_d[b, jt * 128:jt * 128 + jsz, :],
                                  osb[:jsz])
            return emit

        # ---------- attention (software-pipelined over head pairs) ----------
        def emit_attn_item(b, pull):
            attnT = atp.tile([128, KT, N], BF16, tag="attnT", name="attnT")
            pts = {}

            def SS(hp):
                for jt, jsz in jtiles:
                    st2 = psS.tile([128, 2, N], F32, tag="psS", name="st2",
                                   padded_shape=[128, 2, 512])
                    for hh in range(2):
                        pb = hh * 64
                        qa = qkT[pb:pb + 64, hp, b * N:(b + 1) * N]
                        ka = qkT[pb:pb + 64, 6 + hp, b * N:(b + 1) * N]
                        nc.tensor.matmul(
                            st2[:jsz, hh],
                            lhsT=ka[:, jt * 128:jt * 128 + jsz],
                            rhs=qa, start=True, stop=True)
                    pt2 = ptp.tile([128, 2, N], BF16, tag="pt", name="pt2")
                    nc.scalar.activation(pt2[:jsz], st2[:jsz], AF.Exp)
                    nc.vector.tensor_tensor(
                        pt2[:jsz], pt2[:jsz],
                        mask_sb[:jsz, jt].rearrange("p (t n) -> p t n", t=2),
                        op=ALU.mult)
                    pts[(hp, jt)] = pt2

            def PV(hp):
                ot2 = psO.tile([65, 2, N], F32, tag="psO", name="ot2",
                               padded_shape=[65, 2, 512])
                for hh in range(2):
                    h = 2 * hp + hh
                    for jt, jsz in jtiles:
                        nc.tensor.matmul(
                            ot2[:, hh],
                            lhsT=v_sb[:jsz, b, jt, h * 65:h * 65 + 65],
                            rhs=pts[(hp, jt)][:jsz, hh],
                            start=(jt == 0), stop=(jt == 1))
                rc = rcp.tile([1, 2, N], F32, tag="rc", name="rc")
                nc.vector.reciprocal(rc, ot2[64:65])
                bc = bcp.tile([64, 2, N], F32, tag="bc", name="bc")
                nc.gpsimd.partition_broadcast(bc, rc)
                for hh in range(2):
                    h = 2 * hp + hh
                    nc.vector.tensor_tensor(
                        attnT[(h % 2) * 64:(h % 2) * 64 + 64, h // 2, :],
                        ot2[0:64, hh], bc[:, hh], op=ALU.mult)

            SS(0)
            pull(2)
            SS(1)
            pull(1)
            for hp in range(H // 2):
                PV(hp)
                if hp + 2 < H // 2:
                    SS(hp + 2)
                pull(2)
            return attnT

        def emit_rep():
            emit_setup()
            nc.vector.memset(
                v_sb.rearrange("p b j (h e) -> p b j h e",
                               e=65)[:, :, :, :, 64:65], 1.0)

            # filler queue, seeded in dependency-safe order
            fill = deque()
            emitted = [0]

            def add_chunk(c):
                for nt in range(NT_QK):
                    fill.append(unit_qkT(nt, 512 * c))

            def add_v(b):
                for jt, jsz in jtiles:
                    for nch in range(2):
                        fill.append(unit_v(b, jt, jsz, nch))

            add_chunk(0)
            add_v(0)
            add_v(1)
            add_chunk(1)
            add_v(2)
            add_v(3)
            add_v(4)
            add_chunk(2)
            add_v(5)
            add_v(6)
            if M > 1536:
                add_chunk(3)
            add_v(7)
            nq = NT_QK if M > 1536 else 0
            # units that must be emitted before attn(b) starts
            req = {0: NT_QK + 4, 1: NT_QK + 8, 2: 2 * NT_QK + 12,
                   3: 2 * NT_QK + 16, 4: 2 * NT_QK + 20,
                   5: 3 * NT_QK + 24, 6: 3 * NT_QK + 28,
                   7: 3 * NT_QK + nq + 32}

            def pull(n):
                for _ in range(n):
                    if fill:
                        fill.popleft()()
                        emitted[0] += 1

            if bpc == 1:
                pull(10 ** 6)
                at = emit_attn_item(0, lambda n: None)
                unit_outproj(0, 0, 128, at)()
                unit_outproj(0, 1, N - 128, at)()
                return

            for b in range(bpc):
                pull(req[b] - emitted[0])
                at = emit_attn_item(b, pull)
                for jt, jsz in jtiles:
                    fill.append(unit_outproj(b, jt, jsz, at))
            pull(10 ** 6)

        for _rep in range(repeat):
            emit_rep()

    nc.compile()
    return nc


_NC_CACHE = {}


def _get_nc(bpc=BPC, repeat=1):
    key = (bpc, repeat)
    if key not in _NC_CACHE:
        _NC_CACHE[key] = build_nc(bpc, repeat)
    return _NC_CACHE[key]


def kernel(x, W_qkv, scale, W_out, b_out, _trace=False):
    x = np.ascontiguousarray(np.asarray(x, dtype=np.float32))
    W_qkv = np.ascontiguousarray(np.asarray(W_qkv, dtype=np.float32))
    scale = np.ascontiguousarray(np.asarray(scale, dtype=np.float32))
    W_out = np.ascontiguousarray(np.asarray(W_out, dtype=np.float32))
    b_out = np.ascontiguousarray(np.asarray(b_out, dtype=np.float32))

    nc = _get_nc()
    in_maps = [{
        "x": x[c * BPC:(c + 1) * BPC],
        "w_qkv": W_qkv,
        "scale": scale,
        "w_out": W_out,
        "b_out": b_out,
    } for c in range(NCORES)]
    try:
        res = bass_utils.run_bass_kernel_spmd(
            nc, in_maps, core_ids=list(range(NCORES)), trace=_trace)
    except ModuleNotFoundError:
        # axon NTFF profiling hook unavailable in this container
        res = bass_utils.run_bass_kernel_spmd(
            nc, in_maps, core_ids=list(range(NCORES)), trace=False)
    out = np.concatenate([res.results[c]["out"] for c in range(NCORES)], axis=0)
    if _trace:
        return out, res
    return out
